# revision 30
# baseline (speedup 1.0000x reference)
"""MoE-style decoder kernel for Trainium2 (Bass/Tile), 8 NeuronCores.

Problem (nn_DecoderZX): for each of B=2048 samples,
    z0 = z[:, :64]; id_b = argmax(z[:, 64:80])
    logits_b = z0_b @ (amat_w + amat_site[id_b]) + offsets[id_b]
    mu_b = softmax(logits_b) * size_factor_b        # [2048]
    theta = exp(px_r)                               # [2048]

Sharding strategy — expert-parallel with host-side routing:
  * The host computes the nuisance ids (argmax over the last 16 columns of z)
    and groups the batch rows by expert.  Experts are paired (i-th largest
    count with i-th smallest) so each of the 8 cores owns exactly 2 experts:
    every expert's combined weight matrix is read from HBM exactly once
    across the machine (the minimum possible weight traffic).
  * SPMD needs one program for all cores, so the two per-core groups are
    padded to the uniform sizes (S0, S1) = (max count of first-pair members,
    max of second).  Padded columns of the stationary operand are zero, and
    the padded output rows are dropped on the host during unshard.
  * Per-core device inputs: zw [65, R + 2*2048] = transposed latents (with a
    constant-1 feature so the per-expert offsets ride as a weight row) packed
    side by side with both combined expert matrices, streamed as chunked DMAs
    on the SP HWDGE queue whose first chunk covers zt plus the first weight
    chunk (shortest path to the first matmul); sfpx [128, T+16] = per-tile
    size-factor columns + px_r rides the ACT HWDGE queue.
  * Per 128-row tile: 4 matmuls (K=65, moving N=512, fp32) accumulate into
    two separate 2-bank PSUM tiles, so each half of ScalarE's exp (with its
    fused row-sum accum_out) depends on — and starts right after — only its
    own two matmuls.  16 zero-data 64-column warm-up matmuls issued at kernel
    start run the PE through its HAM cold window while the weight DMAs
    stream in, so every real matmul executes at the warm 2.4 GHz clock.
    VectorE computes 1/sum x size_factor and scales/writes the tile in
    column halves.  Row tiles run largest-first so the smallest tile's
    writeout ends the kernel.  Softmax max-subtraction is skipped when a
    rigorous Cauchy-Schwarz bound on max|logit| proves exp cannot overflow
    (softmax is shift-invariant); otherwise a max-subtracting variant runs.
  * theta = exp(px_r) is computed on-device as a [128, 16] tile.
"""

import numpy as np

B = 2048
L = 64  # latent dim
NE = 16  # nuisance categories (experts)
O = 2048  # output dim
NCORES = 8
K = L + 1  # contraction dim incl. the constant-1 bias feature
NCHUNK = 512  # fp32 moving-operand free-dim max on TRN2
P = 128  # partitions / row-tile height
H = O // 2  # softmax column-half width
N_WARMUP_MM = 16  # zero-data 64-col matmuls to warm the PE HAM clock
USE_F32R = False  # experimental: float32r matmul inputs (reduced precision)

_prog_cache: dict = {}
last_results = None  # BassKernelResults of the most recent kernel() call


def _tiling(S0, S1):
    """Row tiles: (row0, nrows, group); tiles never span groups.  Ordered
    largest-first so the smallest tile's writeout ends the kernel."""
    tiles = []
    for g, (gstart, gsize) in enumerate(((0, S0), (S0, S1))):
        for r in range(0, gsize, P):
            tiles.append((gstart + r, min(P, gsize - r), g))
    tiles.sort(key=lambda t: -t[1])
    return tiles


def _build_program(S0, S1, use_max):
    import concourse.bacc as bacc
    import concourse.tile as tile
    from concourse import mybir

    f32 = mybir.dt.float32
    f32mm = mybir.dt.float32r if USE_F32R else f32
    Exp = mybir.ActivationFunctionType.Exp
    mult = mybir.AluOpType.mult
    add = mybir.AluOpType.add

    R = S0 + S1
    tiles = _tiling(S0, S1)
    T = len(tiles)
    total_cols = R + 2 * O

    nc = bacc.Bacc("TRN2", target_bir_lowering=False, name="moe_decoder")
    zw_d = nc.dram_tensor("zw", [K, total_cols], f32mm, kind="ExternalInput")
    sfpx_d = nc.dram_tensor("sfpx", [P, T + O // P], f32, kind="ExternalInput")
    mu_d = nc.dram_tensor("mu", [R, O], f32, kind="ExternalOutput")
    theta_d = nc.dram_tensor("theta", [P, O // P], f32, kind="ExternalOutput")

    with tile.TileContext(nc) as tc:
        with (
            tc.tile_pool(name="singles", bufs=1) as singles,
            tc.tile_pool(name="psum", bufs=4, space="PSUM") as psum_pool,
            tc.tile_pool(name="work", bufs=3) as work,
            tc.tile_pool(name="small", bufs=3) as small,
        ):
            # PE warm-up: zero-data matmuls with no input dependency keep the
            # HAM activity window busy while the weight DMAs stream in.
            warm = singles.tile([P, P], f32, tag="warm")
            nc.gpsimd.memset(warm, 0.0)
            for _ in range(N_WARMUP_MM):
                pw = psum_pool.tile([P, H], f32, tag="psum")
                nc.tensor.matmul(
                    pw[:64, 0:64],
                    warm[:, 0:64],
                    warm[:, 64:128],
                    start=True,
                    stop=True,
                )

            # Packed input: [zt | w0 | w1].  First chunk carries zt + the
            # first 512 weight columns so matmul 0 starts as early as
            # possible; both HWDGE queues (SP + ACT) share the issue load.
            zw_sb = singles.tile([K, total_cols], f32mm, tag="zw")
            bounds = [0, R + NCHUNK]
            while bounds[-1] < total_cols:
                bounds.append(min(bounds[-1] + NCHUNK, total_cols))
            for ci in range(len(bounds) - 1):
                lo, hi = bounds[ci], bounds[ci + 1]
                nc.sync.dma_start(out=zw_sb[:, lo:hi], in_=zw_d[:, lo:hi])
            zt_sb = zw_sb[:, 0:R]
            w_sb = [zw_sb[:, R + g * O : R + (g + 1) * O] for g in range(2)]

            sfpx_sb = singles.tile([P, T + O // P], f32, tag="sfpx")
            nc.scalar.dma_start(out=sfpx_sb, in_=sfpx_d[:, :])
            sf_sb = sfpx_sb[:, 0:T]
            pxr_sb = sfpx_sb[:, T : T + O // P]

            # theta = exp(px_r), computed once (identical on every core)
            th_sb = singles.tile([P, O // P], f32, tag="th")
            nc.scalar.activation(out=th_sb, in_=pxr_sb, func=Exp)
            nc.scalar.dma_start(out=theta_d[:, :], in_=th_sb)

            for t, (r0, p, g) in enumerate(tiles):
                # two separate 2-bank PSUM tiles per row tile so each exp
                # half's RAW dependency covers only its own two matmuls
                psA = psum_pool.tile([P, H], f32, tag="psum")
                psB = psum_pool.tile([P, H], f32, tag="psum")
                ps = [psA, psB]
                for ci, n0 in enumerate(range(0, O, NCHUNK)):
                    nc.tensor.matmul(
                        ps[ci // 2][:p, (ci % 2) * NCHUNK : (ci % 2 + 1) * NCHUNK],
                        zt_sb[:, r0 : r0 + p],
                        w_sb[g][:, n0 : n0 + NCHUNK],
                        start=True,
                        stop=True,
                    )
                e_t = work.tile([P, O], f32, tag="e")
                if use_max:
                    # safe path: per-half max subtraction via the combined max
                    m0 = small.tile([P, 1], f32, tag="m0")
                    m1 = small.tile([P, 1], f32, tag="m1")
                    nc.vector.reduce_max(
                        m0[:p], ps[0][:p], axis=mybir.AxisListType.X, negate=True
                    )
                    nc.vector.reduce_max(
                        m1[:p], ps[1][:p], axis=mybir.AxisListType.X, negate=True
                    )
                    rowmax = small.tile([P, 1], f32, tag="rowmax")
                    nc.vector.tensor_tensor(
                        out=rowmax[:p], in0=m0[:p], in1=m1[:p],
                        op=mybir.AluOpType.min,
                    )
                    s0 = small.tile([P, 1], f32, tag="s0")
                    s1 = small.tile([P, 1], f32, tag="s1")
                    nc.scalar.activation(
                        out=e_t[:p, 0:H], in_=ps[0][:p], func=Exp,
                        bias=rowmax[:p], accum_out=s0[:p],
                    )
                    nc.scalar.activation(
                        out=e_t[:p, H:O], in_=ps[1][:p], func=Exp,
                        bias=rowmax[:p], accum_out=s1[:p],
                    )
                    rowsum = small.tile([P, 1], f32, tag="rowsum")
                    nc.vector.tensor_tensor(
                        out=rowsum[:p], in0=s0[:p], in1=s1[:p], op=add
                    )
                else:
                    # exp in column halves: each half starts right after its
                    # own two matmuls
                    s0 = small.tile([P, 1], f32, tag="s0")
                    s1 = small.tile([P, 1], f32, tag="s1")
                    nc.scalar.activation(
                        out=e_t[:p, 0:H], in_=ps[0][:p], func=Exp,
                        accum_out=s0[:p],
                    )
                    nc.scalar.activation(
                        out=e_t[:p, H:O], in_=ps[1][:p], func=Exp,
                        accum_out=s1[:p],
                    )
                    rowsum = small.tile([P, 1], f32, tag="rowsum")
                    nc.vector.tensor_tensor(
                        out=rowsum[:p], in0=s0[:p], in1=s1[:p], op=add
                    )
                rcp = small.tile([P, 1], f32, tag="rcp")
                nc.vector.reciprocal(rcp[:p], rowsum[:p])
                scale = small.tile([P, 1], f32, tag="scale")
                nc.vector.tensor_tensor(
                    out=scale[:p], in0=rcp[:p], in1=sf_sb[:p, t : t + 1], op=mult
                )
                o_t = work.tile([P, O], f32, tag="o")
                for hi_, h0 in enumerate((0, H)):
                    nc.vector.tensor_scalar(
                        out=o_t[:p, h0 : h0 + H],
                        in0=e_t[:p, h0 : h0 + H],
                        scalar1=scale[:p],
                        scalar2=None,
                        op0=mult,
                    )
                    nc.sync.dma_start(
                        out=mu_d[r0 : r0 + p, h0 : h0 + H],
                        in_=o_t[:p, h0 : h0 + H],
                    )

    nc.compile()
    return nc, tiles


def prepare(z, size_factor, amat_w, amat_site, offsets, px_r):
    """Host-side routing + packing.  Returns (nc, in_maps, meta) where meta
    carries what unshard() needs."""
    z = np.ascontiguousarray(np.asarray(z, dtype=np.float32))
    size_factor = np.asarray(size_factor, dtype=np.float32).reshape(B)
    amat_w = np.asarray(amat_w, dtype=np.float32)
    amat_site = np.asarray(amat_site, dtype=np.float32)
    offsets = np.asarray(offsets, dtype=np.float32)
    px_r = np.asarray(px_r, dtype=np.float32)

    z0 = z[:, :L]
    ids = np.argmax(z[:, L:], axis=1)
    counts = np.bincount(ids, minlength=NE)

    # Pair experts: i-th largest with i-th smallest -> balanced core loads.
    order = np.argsort(-counts, kind="stable")
    pairs = [(int(order[i]), int(order[2 * NCORES - 1 - i])) for i in range(NCORES)]
    S0 = max(1, int(max(counts[a] for a, _ in pairs)))
    S1 = max(1, int(max(counts[b] for _, b in pairs)))
    R = S0 + S1

    # Combined per-expert weights [NE, K, O]: rows 0..63 = amat_w+amat_site[e],
    # row 64 = offsets[e] (activated by the constant-1 feature).
    w_all = np.empty((NE, K, O), dtype=np.float32)
    w_all[:, :L, :] = amat_w[None, :, :] + amat_site
    w_all[:, L, :] = offsets

    # Overflow safety: |logit| <= max_b ||[z0_b;1]|| * max_{e,o} ||W[e,:,o]||.
    # exp is finite in f32 for logits < 85; otherwise subtract the row max.
    zmax = float(np.sqrt((z0 * z0).sum(axis=1).max() + 1.0))
    wmax = float(np.sqrt((w_all * w_all).sum(axis=1).max()))
    use_max = zmax * wmax > 85.0

    key = (S0, S1, use_max, USE_F32R)
    if key not in _prog_cache:
        _prog_cache[key] = _build_program(S0, S1, use_max)
    nc, tiles = _prog_cache[key]
    T = len(tiles)
    total_cols = R + 2 * O

    in_maps = []
    core_rows = []
    pxr_in = px_r.reshape(P, O // P)
    for c in range(NCORES):
        ea, eb = pairs[c]
        rows0 = np.where(ids == ea)[0]
        rows1 = np.where(ids == eb)[0]
        core_rows.append((rows0, rows1))

        zw = np.zeros((K, total_cols), dtype=np.float32)
        zw[:L, : len(rows0)] = z0[rows0].T
        zw[L, : len(rows0)] = 1.0
        zw[:L, S0 : S0 + len(rows1)] = z0[rows1].T
        zw[L, S0 : S0 + len(rows1)] = 1.0
        zw[:, R : R + O] = w_all[ea]
        zw[:, R + O : R + 2 * O] = w_all[eb]

        sf_pad = np.ones(R, dtype=np.float32)
        sf_pad[: len(rows0)] = size_factor[rows0]
        sf_pad[S0 : S0 + len(rows1)] = size_factor[rows1]
        sfpx = np.ones((P, T + O // P), dtype=np.float32)
        for t, (r0, p, _g) in enumerate(tiles):
            sfpx[:p, t] = sf_pad[r0 : r0 + p]
        sfpx[:, T:] = pxr_in

        in_maps.append({"zw": zw, "sfpx": sfpx})

    return nc, in_maps, (core_rows, S0)


def unshard(results, meta):
    core_rows, S0 = meta
    mu = np.empty((B, O), dtype=np.float32)
    for c in range(NCORES):
        rows0, rows1 = core_rows[c]
        mu_c = results[c]["mu"]
        mu[rows0] = mu_c[: len(rows0)]
        mu[rows1] = mu_c[S0 : S0 + len(rows1)]
    theta = np.ascontiguousarray(results[0]["theta"].reshape(O))
    return mu, theta


def _ensure_axon_hooks_stub():
    """run_bass_kernel_spmd(trace=True) under axon imports antenv.axon_hooks,
    which some containers lack.  Provide a no-hook stub (only if the module is
    genuinely absent) so a BASS_TRACE=1 environment degrades to an untraced
    run instead of crashing."""
    import importlib.util
    import sys
    import types

    try:
        if importlib.util.find_spec("antenv.axon_hooks") is not None:
            return
    except (ImportError, ModuleNotFoundError):
        pass
    if "antenv.axon_hooks" in sys.modules:
        return
    m = types.ModuleType("antenv.axon_hooks")
    m.get_axon_ntff_profile_hook = lambda: None
    sys.modules["antenv.axon_hooks"] = m
    try:
        import antenv

        if not hasattr(antenv, "axon_hooks"):
            antenv.axon_hooks = m
    except ImportError:
        pass


def kernel(z, size_factor, amat_w, amat_site, offsets, px_r):
    from concourse import bass_utils

    _ensure_axon_hooks_stub()
    nc, in_maps, meta = prepare(z, size_factor, amat_w, amat_site, offsets, px_r)
    res = bass_utils.run_bass_kernel_spmd(nc, in_maps, core_ids=list(range(NCORES)))
    global last_results
    last_results = res
    return unshard(res.results, meta)


# revision 31
# speedup vs baseline: 1.0030x; 1.0030x over previous
"""MoE-style decoder kernel for Trainium2 (Bass/Tile), 8 NeuronCores.

Problem (nn_DecoderZX): for each of B=2048 samples,
    z0 = z[:, :64]; id_b = argmax(z[:, 64:80])
    logits_b = z0_b @ (amat_w + amat_site[id_b]) + offsets[id_b]
    mu_b = softmax(logits_b) * size_factor_b        # [2048]
    theta = exp(px_r)                               # [2048]

Sharding strategy — expert-parallel with host-side routing:
  * The host computes the nuisance ids (argmax over the last 16 columns of z)
    and groups the batch rows by expert.  Experts are paired (i-th largest
    count with i-th smallest) so each of the 8 cores owns exactly 2 experts:
    every expert's combined weight matrix is read from HBM exactly once
    across the machine (the minimum possible weight traffic).
  * SPMD needs one program for all cores, so the two per-core groups are
    padded to the uniform sizes (S0, S1) = (max count of first-pair members,
    max of second).  Padded columns of the stationary operand are zero, and
    the padded output rows are dropped on the host during unshard.
  * Per-core device inputs: zw [65, R + 2*2048] = transposed latents (with a
    constant-1 feature so the per-expert offsets ride as a weight row) packed
    side by side with both combined expert matrices, streamed as chunked DMAs
    on the SP HWDGE queue whose first chunk covers zt plus the first weight
    chunk (shortest path to the first matmul); sfpx [128, T+16] = per-tile
    size-factor columns + px_r rides the ACT HWDGE queue.
  * Per 128-row tile: 4 matmuls (K=65, moving N=512, fp32) accumulate into
    two separate 2-bank PSUM tiles, so each half of ScalarE's exp (with its
    fused row-sum accum_out) depends on — and starts right after — only its
    own two matmuls.  16 zero-data 64-column warm-up matmuls issued at kernel
    start run the PE through its HAM cold window while the weight DMAs
    stream in, so every real matmul executes at the warm 2.4 GHz clock.
    VectorE computes 1/sum x size_factor and scales/writes the tile in
    column halves.  Row tiles run largest-first so the smallest tile's
    writeout ends the kernel.  Softmax max-subtraction is skipped when a
    rigorous Cauchy-Schwarz bound on max|logit| proves exp cannot overflow
    (softmax is shift-invariant); otherwise a max-subtracting variant runs.
  * theta = exp(px_r) is computed on-device as a [128, 16] tile.
"""

import numpy as np

B = 2048
L = 64  # latent dim
NE = 16  # nuisance categories (experts)
O = 2048  # output dim
NCORES = 8
K = L + 1  # contraction dim incl. the constant-1 bias feature
NCHUNK = 512  # fp32 moving-operand free-dim max on TRN2
P = 128  # partitions / row-tile height
H = O // 2  # softmax column-half width
N_WARMUP_MM = 16  # zero-data 64-col matmuls to warm the PE HAM clock
USE_F32R = False  # experimental: float32r matmul inputs (reduced precision)

_prog_cache: dict = {}
last_results = None  # BassKernelResults of the most recent kernel() call


def _tiling(S0, S1):
    """Row tiles: (row0, nrows, group); tiles never span groups.  Ordered
    largest-first so the smallest tile's writeout ends the kernel."""
    tiles = []
    for g, (gstart, gsize) in enumerate(((0, S0), (S0, S1))):
        for r in range(0, gsize, P):
            tiles.append((gstart + r, min(P, gsize - r), g))
    tiles.sort(key=lambda t: -t[1])
    return tiles


def _build_program(S0, S1, use_max):
    import concourse.bacc as bacc
    import concourse.tile as tile
    from concourse import mybir

    f32 = mybir.dt.float32
    f32mm = mybir.dt.float32r if USE_F32R else f32
    Exp = mybir.ActivationFunctionType.Exp
    mult = mybir.AluOpType.mult
    add = mybir.AluOpType.add

    R = S0 + S1
    tiles = _tiling(S0, S1)
    T = len(tiles)
    total_cols = R + 2 * O

    nc = bacc.Bacc("TRN2", target_bir_lowering=False, name="moe_decoder")
    zw_d = nc.dram_tensor("zw", [K, total_cols], f32mm, kind="ExternalInput")
    sfpx_d = nc.dram_tensor("sfpx", [P, T + O // P], f32, kind="ExternalInput")
    mu_d = nc.dram_tensor("mu", [R, O], f32, kind="ExternalOutput")
    theta_d = nc.dram_tensor("theta", [P, O // P], f32, kind="ExternalOutput")

    with tile.TileContext(nc) as tc:
        with (
            tc.tile_pool(name="singles", bufs=1) as singles,
            tc.tile_pool(name="psum", bufs=4, space="PSUM") as psum_pool,
            tc.tile_pool(name="work", bufs=3) as work,
            tc.tile_pool(name="small", bufs=3) as small,
        ):
            # PE warm-up: zero-data matmuls with no input dependency keep the
            # HAM activity window busy while the weight DMAs stream in.
            warm = singles.tile([P, P], f32, tag="warm")
            nc.gpsimd.memset(warm, 0.0)
            for _ in range(N_WARMUP_MM):
                pw = psum_pool.tile([P, H], f32, tag="psum")
                nc.tensor.matmul(
                    pw[:64, 0:64],
                    warm[:, 0:64],
                    warm[:, 64:128],
                    start=True,
                    stop=True,
                )

            # Packed input: [zt | w0 | w1].  First chunk carries zt + the
            # first 512 weight columns so matmul 0 starts as early as
            # possible; both HWDGE queues (SP + ACT) share the issue load.
            zw_sb = singles.tile([K, total_cols], f32mm, tag="zw")
            bounds = [0, R + NCHUNK]
            while bounds[-1] < total_cols:
                bounds.append(min(bounds[-1] + NCHUNK, total_cols))
            for ci in range(len(bounds) - 1):
                lo, hi = bounds[ci], bounds[ci + 1]
                nc.sync.dma_start(out=zw_sb[:, lo:hi], in_=zw_d[:, lo:hi])
            zt_sb = zw_sb[:, 0:R]
            w_sb = [zw_sb[:, R + g * O : R + (g + 1) * O] for g in range(2)]

            sfpx_sb = singles.tile([P, T + O // P], f32, tag="sfpx")
            nc.scalar.dma_start(out=sfpx_sb, in_=sfpx_d[:, :])
            sf_sb = sfpx_sb[:, 0:T]
            pxr_sb = sfpx_sb[:, T : T + O // P]

            # theta = exp(px_r), computed once (identical on every core)
            th_sb = singles.tile([P, O // P], f32, tag="th")
            nc.scalar.activation(out=th_sb, in_=pxr_sb, func=Exp)
            nc.scalar.dma_start(out=theta_d[:, :], in_=th_sb)

            for t, (r0, p, g) in enumerate(tiles):
                # two separate 2-bank PSUM tiles per row tile so each exp
                # half's RAW dependency covers only its own two matmuls
                psA = psum_pool.tile([P, H], f32, tag="psum")
                psB = psum_pool.tile([P, H], f32, tag="psum")
                ps = [psA, psB]
                for ci, n0 in enumerate(range(0, O, NCHUNK)):
                    nc.tensor.matmul(
                        ps[ci // 2][:p, (ci % 2) * NCHUNK : (ci % 2 + 1) * NCHUNK],
                        zt_sb[:, r0 : r0 + p],
                        w_sb[g][:, n0 : n0 + NCHUNK],
                        start=True,
                        stop=True,
                    )
                e_t = work.tile([P, O], f32, tag="e")
                if use_max:
                    # safe path: per-half max subtraction via the combined max
                    m0 = small.tile([P, 1], f32, tag="m0")
                    m1 = small.tile([P, 1], f32, tag="m1")
                    nc.vector.reduce_max(
                        m0[:p], ps[0][:p], axis=mybir.AxisListType.X, negate=True
                    )
                    nc.vector.reduce_max(
                        m1[:p], ps[1][:p], axis=mybir.AxisListType.X, negate=True
                    )
                    rowmax = small.tile([P, 1], f32, tag="rowmax")
                    nc.vector.tensor_tensor(
                        out=rowmax[:p], in0=m0[:p], in1=m1[:p],
                        op=mybir.AluOpType.min,
                    )
                    s0 = small.tile([P, 1], f32, tag="s0")
                    s1 = small.tile([P, 1], f32, tag="s1")
                    nc.scalar.activation(
                        out=e_t[:p, 0:H], in_=ps[0][:p], func=Exp,
                        bias=rowmax[:p], accum_out=s0[:p],
                    )
                    nc.scalar.activation(
                        out=e_t[:p, H:O], in_=ps[1][:p], func=Exp,
                        bias=rowmax[:p], accum_out=s1[:p],
                    )
                    rowsum = small.tile([P, 1], f32, tag="rowsum")
                    nc.vector.tensor_tensor(
                        out=rowsum[:p], in0=s0[:p], in1=s1[:p], op=add
                    )
                else:
                    # exp in column halves: each half starts right after its
                    # own two matmuls
                    s0 = small.tile([P, 1], f32, tag="s0")
                    s1 = small.tile([P, 1], f32, tag="s1")
                    nc.scalar.activation(
                        out=e_t[:p, 0:H], in_=ps[0][:p], func=Exp,
                        accum_out=s0[:p],
                    )
                    nc.scalar.activation(
                        out=e_t[:p, H:O], in_=ps[1][:p], func=Exp,
                        accum_out=s1[:p],
                    )
                    rowsum = small.tile([P, 1], f32, tag="rowsum")
                    nc.vector.tensor_tensor(
                        out=rowsum[:p], in0=s0[:p], in1=s1[:p], op=add
                    )
                rcp = small.tile([P, 1], f32, tag="rcp")
                nc.vector.reciprocal(rcp[:p], rowsum[:p])
                o_t = work.tile([P, O], f32, tag="o")
                for hi_, h0 in enumerate((0, H)):
                    # fused (e * 1/sum) * sf in one DVE pass
                    nc.vector.tensor_scalar(
                        out=o_t[:p, h0 : h0 + H],
                        in0=e_t[:p, h0 : h0 + H],
                        scalar1=rcp[:p],
                        scalar2=sf_sb[:p, t : t + 1],
                        op0=mult,
                        op1=mult,
                    )
                    nc.sync.dma_start(
                        out=mu_d[r0 : r0 + p, h0 : h0 + H],
                        in_=o_t[:p, h0 : h0 + H],
                    )

    nc.compile()
    return nc, tiles


def prepare(z, size_factor, amat_w, amat_site, offsets, px_r):
    """Host-side routing + packing.  Returns (nc, in_maps, meta) where meta
    carries what unshard() needs."""
    z = np.ascontiguousarray(np.asarray(z, dtype=np.float32))
    size_factor = np.asarray(size_factor, dtype=np.float32).reshape(B)
    amat_w = np.asarray(amat_w, dtype=np.float32)
    amat_site = np.asarray(amat_site, dtype=np.float32)
    offsets = np.asarray(offsets, dtype=np.float32)
    px_r = np.asarray(px_r, dtype=np.float32)

    z0 = z[:, :L]
    ids = np.argmax(z[:, L:], axis=1)
    counts = np.bincount(ids, minlength=NE)

    # Pair experts: i-th largest with i-th smallest -> balanced core loads.
    order = np.argsort(-counts, kind="stable")
    pairs = [(int(order[i]), int(order[2 * NCORES - 1 - i])) for i in range(NCORES)]
    S0 = max(1, int(max(counts[a] for a, _ in pairs)))
    S1 = max(1, int(max(counts[b] for _, b in pairs)))
    R = S0 + S1

    # Combined per-expert weights [NE, K, O]: rows 0..63 = amat_w+amat_site[e],
    # row 64 = offsets[e] (activated by the constant-1 feature).
    w_all = np.empty((NE, K, O), dtype=np.float32)
    w_all[:, :L, :] = amat_w[None, :, :] + amat_site
    w_all[:, L, :] = offsets

    # Overflow safety: |logit| <= max_b ||[z0_b;1]|| * max_{e,o} ||W[e,:,o]||.
    # exp is finite in f32 for logits < 85; otherwise subtract the row max.
    zmax = float(np.sqrt((z0 * z0).sum(axis=1).max() + 1.0))
    wmax = float(np.sqrt((w_all * w_all).sum(axis=1).max()))
    use_max = zmax * wmax > 85.0

    key = (S0, S1, use_max, USE_F32R)
    if key not in _prog_cache:
        _prog_cache[key] = _build_program(S0, S1, use_max)
    nc, tiles = _prog_cache[key]
    T = len(tiles)
    total_cols = R + 2 * O

    in_maps = []
    core_rows = []
    pxr_in = px_r.reshape(P, O // P)
    for c in range(NCORES):
        ea, eb = pairs[c]
        rows0 = np.where(ids == ea)[0]
        rows1 = np.where(ids == eb)[0]
        core_rows.append((rows0, rows1))

        zw = np.zeros((K, total_cols), dtype=np.float32)
        zw[:L, : len(rows0)] = z0[rows0].T
        zw[L, : len(rows0)] = 1.0
        zw[:L, S0 : S0 + len(rows1)] = z0[rows1].T
        zw[L, S0 : S0 + len(rows1)] = 1.0
        zw[:, R : R + O] = w_all[ea]
        zw[:, R + O : R + 2 * O] = w_all[eb]

        sf_pad = np.ones(R, dtype=np.float32)
        sf_pad[: len(rows0)] = size_factor[rows0]
        sf_pad[S0 : S0 + len(rows1)] = size_factor[rows1]
        sfpx = np.ones((P, T + O // P), dtype=np.float32)
        for t, (r0, p, _g) in enumerate(tiles):
            sfpx[:p, t] = sf_pad[r0 : r0 + p]
        sfpx[:, T:] = pxr_in

        in_maps.append({"zw": zw, "sfpx": sfpx})

    return nc, in_maps, (core_rows, S0)


def unshard(results, meta):
    core_rows, S0 = meta
    mu = np.empty((B, O), dtype=np.float32)
    for c in range(NCORES):
        rows0, rows1 = core_rows[c]
        mu_c = results[c]["mu"]
        mu[rows0] = mu_c[: len(rows0)]
        mu[rows1] = mu_c[S0 : S0 + len(rows1)]
    theta = np.ascontiguousarray(results[0]["theta"].reshape(O))
    return mu, theta


def _ensure_axon_hooks_stub():
    """run_bass_kernel_spmd(trace=True) under axon imports antenv.axon_hooks,
    which some containers lack.  Provide a no-hook stub (only if the module is
    genuinely absent) so a BASS_TRACE=1 environment degrades to an untraced
    run instead of crashing."""
    import importlib.util
    import sys
    import types

    try:
        if importlib.util.find_spec("antenv.axon_hooks") is not None:
            return
    except (ImportError, ModuleNotFoundError):
        pass
    if "antenv.axon_hooks" in sys.modules:
        return
    m = types.ModuleType("antenv.axon_hooks")
    m.get_axon_ntff_profile_hook = lambda: None
    sys.modules["antenv.axon_hooks"] = m
    try:
        import antenv

        if not hasattr(antenv, "axon_hooks"):
            antenv.axon_hooks = m
    except ImportError:
        pass


def kernel(z, size_factor, amat_w, amat_site, offsets, px_r):
    from concourse import bass_utils

    _ensure_axon_hooks_stub()
    nc, in_maps, meta = prepare(z, size_factor, amat_w, amat_site, offsets, px_r)
    res = bass_utils.run_bass_kernel_spmd(nc, in_maps, core_ids=list(range(NCORES)))
    global last_results
    last_results = res
    return unshard(res.results, meta)


# revision 32
# speedup vs baseline: 1.0331x; 1.0300x over previous
"""MoE-style decoder kernel for Trainium2 (Bass/Tile), 8 NeuronCores.

Problem (nn_DecoderZX): for each of B=2048 samples,
    z0 = z[:, :64]; id_b = argmax(z[:, 64:80])
    logits_b = z0_b @ (amat_w + amat_site[id_b]) + offsets[id_b]
    mu_b = softmax(logits_b) * size_factor_b        # [2048]
    theta = exp(px_r)                               # [2048]

Sharding strategy — expert-parallel with host-side routing:
  * The host computes the nuisance ids (argmax over the last 16 columns of z)
    and groups the batch rows by expert.  Experts are paired (i-th largest
    count with i-th smallest) so each of the 8 cores owns exactly 2 experts:
    every expert's combined weight matrix is read from HBM exactly once
    across the machine (the minimum possible weight traffic).
  * SPMD needs one program for all cores, so the two per-core groups are
    padded to the uniform sizes (S0, S1) = (max count of first-pair members,
    max of second).  Padded columns of the stationary operand are zero, and
    the padded output rows are dropped on the host during unshard.
  * Per-core device inputs: zw [65, R + 2*2048] = transposed latents (with a
    constant-1 feature so the per-expert offsets ride as a weight row) packed
    side by side with both combined expert matrices, streamed as chunked DMAs
    on the SP HWDGE queue whose first chunk covers zt plus the first weight
    chunk (shortest path to the first matmul); sfpx [128, T+16] = per-tile
    size-factor columns + px_r rides the ACT HWDGE queue.
  * Per 128-row tile: 4 matmuls (K=65, moving N=512, fp32) accumulate into
    two separate 2-bank PSUM tiles, so each half of ScalarE's exp (with its
    fused row-sum accum_out) depends on — and starts right after — only its
    own two matmuls.  16 zero-data 64-column warm-up matmuls issued at kernel
    start run the PE through its HAM cold window while the weight DMAs
    stream in, so every real matmul executes at the warm 2.4 GHz clock.
    VectorE computes 1/sum x size_factor and scales/writes the tile in
    column halves.  Row tiles run largest-first so the smallest tile's
    writeout ends the kernel.  Softmax max-subtraction is skipped when a
    rigorous Cauchy-Schwarz bound on max|logit| proves exp cannot overflow
    (softmax is shift-invariant); otherwise a max-subtracting variant runs.
  * theta = exp(px_r) is computed on-device as a [128, 16] tile.
"""

import numpy as np

B = 2048
L = 64  # latent dim
NE = 16  # nuisance categories (experts)
O = 2048  # output dim
NCORES = 8
K = L + 1  # contraction dim incl. the constant-1 bias feature
NCHUNK = 512  # fp32 moving-operand free-dim max on TRN2
P = 128  # partitions / row-tile height
H = O // 2  # softmax column-half width
N_WARMUP_MM = 12  # zero-data 64-col matmuls to pre-warm the PE HAM clock
USE_F32R = False  # experimental: float32r matmul inputs (reduced precision)

_prog_cache: dict = {}
last_results = None  # BassKernelResults of the most recent kernel() call


def _tiling(S0, S1):
    """Row tiles: (row0, nrows, group); tiles never span groups.  Ordered
    largest-first so the smallest tile's writeout ends the kernel."""
    tiles = []
    for g, (gstart, gsize) in enumerate(((0, S0), (S0, S1))):
        for r in range(0, gsize, P):
            tiles.append((gstart + r, min(P, gsize - r), g))
    tiles.sort(key=lambda t: -t[1])
    return tiles


def _build_program(S0, S1, use_max):
    import concourse.bacc as bacc
    import concourse.tile as tile
    from concourse import mybir

    f32 = mybir.dt.float32
    f32mm = mybir.dt.float32r if USE_F32R else f32
    Exp = mybir.ActivationFunctionType.Exp
    mult = mybir.AluOpType.mult
    add = mybir.AluOpType.add

    R = S0 + S1
    tiles = _tiling(S0, S1)
    T = len(tiles)
    total_cols = R + 2 * O

    nc = bacc.Bacc("TRN2", target_bir_lowering=False, name="moe_decoder")
    zw_d = nc.dram_tensor("zw", [K, total_cols], f32mm, kind="ExternalInput")
    sfpx_d = nc.dram_tensor("sfpx", [P, T + O // P], f32, kind="ExternalInput")
    mu_d = nc.dram_tensor("mu", [R, O], f32, kind="ExternalOutput")
    theta_d = nc.dram_tensor("theta", [P, O // P], f32, kind="ExternalOutput")

    with tile.TileContext(nc) as tc:
        with (
            tc.tile_pool(name="singles", bufs=1) as singles,
            tc.tile_pool(name="psum", bufs=4, space="PSUM") as psum_pool,
            tc.tile_pool(name="work", bufs=3) as work,
            tc.tile_pool(name="small", bufs=3) as small,
        ):
            # PE warm-up: zero-data matmuls with no input dependency keep the
            # HAM activity window busy while the weight DMAs stream in.
            warm = singles.tile([P, P], f32, tag="warm")
            nc.gpsimd.memset(warm, 0.0)
            for _ in range(N_WARMUP_MM):
                pw = psum_pool.tile([P, H], f32, tag="psum")
                nc.tensor.matmul(
                    pw[:64, 0:64],
                    warm[:, 0:64],
                    warm[:, 64:128],
                    start=True,
                    stop=True,
                )

            # Packed input: [zt | w0 | w1].  First chunk carries zt + the
            # first 512 weight columns so matmul 0 starts as early as
            # possible; both HWDGE queues (SP + ACT) share the issue load.
            zw_sb = singles.tile([K, total_cols], f32mm, tag="zw")
            bounds = [0, R + NCHUNK]
            while bounds[-1] < total_cols:
                bounds.append(min(bounds[-1] + NCHUNK, total_cols))
            for ci in range(len(bounds) - 1):
                lo, hi = bounds[ci], bounds[ci + 1]
                nc.sync.dma_start(out=zw_sb[:, lo:hi], in_=zw_d[:, lo:hi])
            zt_sb = zw_sb[:, 0:R]
            w_sb = [zw_sb[:, R + g * O : R + (g + 1) * O] for g in range(2)]

            sfpx_sb = singles.tile([P, T + O // P], f32, tag="sfpx")
            nc.scalar.dma_start(out=sfpx_sb, in_=sfpx_d[:, :])
            sf_sb = sfpx_sb[:, 0:T]
            pxr_sb = sfpx_sb[:, T : T + O // P]

            # theta = exp(px_r), computed once (identical on every core)
            th_sb = singles.tile([P, O // P], f32, tag="th")
            nc.scalar.activation(out=th_sb, in_=pxr_sb, func=Exp)
            nc.scalar.dma_start(out=theta_d[:, :], in_=th_sb)

            for t, (r0, p, g) in enumerate(tiles):
                # two separate 2-bank PSUM tiles per row tile so each exp
                # half's RAW dependency covers only its own two matmuls
                psA = psum_pool.tile([P, H], f32, tag="psum")
                psB = psum_pool.tile([P, H], f32, tag="psum")
                ps = [psA, psB]
                for ci, n0 in enumerate(range(0, O, NCHUNK)):
                    nc.tensor.matmul(
                        ps[ci // 2][:p, (ci % 2) * NCHUNK : (ci % 2 + 1) * NCHUNK],
                        zt_sb[:, r0 : r0 + p],
                        w_sb[g][:, n0 : n0 + NCHUNK],
                        start=True,
                        stop=True,
                    )
                e_t = work.tile([P, O], f32, tag="e")
                if use_max:
                    # safe path: per-half max subtraction via the combined max
                    m0 = small.tile([P, 1], f32, tag="m0")
                    m1 = small.tile([P, 1], f32, tag="m1")
                    nc.vector.reduce_max(
                        m0[:p], ps[0][:p], axis=mybir.AxisListType.X, negate=True
                    )
                    nc.vector.reduce_max(
                        m1[:p], ps[1][:p], axis=mybir.AxisListType.X, negate=True
                    )
                    rowmax = small.tile([P, 1], f32, tag="rowmax")
                    nc.vector.tensor_tensor(
                        out=rowmax[:p], in0=m0[:p], in1=m1[:p],
                        op=mybir.AluOpType.min,
                    )
                    s0 = small.tile([P, 1], f32, tag="s0")
                    s1 = small.tile([P, 1], f32, tag="s1")
                    nc.scalar.activation(
                        out=e_t[:p, 0:H], in_=ps[0][:p], func=Exp,
                        bias=rowmax[:p], accum_out=s0[:p],
                    )
                    nc.scalar.activation(
                        out=e_t[:p, H:O], in_=ps[1][:p], func=Exp,
                        bias=rowmax[:p], accum_out=s1[:p],
                    )
                    rowsum = small.tile([P, 1], f32, tag="rowsum")
                    nc.vector.tensor_tensor(
                        out=rowsum[:p], in0=s0[:p], in1=s1[:p], op=add
                    )
                else:
                    # exp in column halves: each half starts right after its
                    # own two matmuls
                    s0 = small.tile([P, 1], f32, tag="s0")
                    s1 = small.tile([P, 1], f32, tag="s1")
                    nc.scalar.activation(
                        out=e_t[:p, 0:H], in_=ps[0][:p], func=Exp,
                        accum_out=s0[:p],
                    )
                    nc.scalar.activation(
                        out=e_t[:p, H:O], in_=ps[1][:p], func=Exp,
                        accum_out=s1[:p],
                    )
                    rowsum = small.tile([P, 1], f32, tag="rowsum")
                    nc.vector.tensor_tensor(
                        out=rowsum[:p], in0=s0[:p], in1=s1[:p], op=add
                    )
                rcp = small.tile([P, 1], f32, tag="rcp")
                nc.vector.reciprocal(rcp[:p], rowsum[:p])
                o_t = work.tile([P, O], f32, tag="o")
                for hi_, h0 in enumerate((0, H)):
                    # fused (e * 1/sum) * sf in one DVE pass
                    nc.vector.tensor_scalar(
                        out=o_t[:p, h0 : h0 + H],
                        in0=e_t[:p, h0 : h0 + H],
                        scalar1=rcp[:p],
                        scalar2=sf_sb[:p, t : t + 1],
                        op0=mult,
                        op1=mult,
                    )
                    nc.sync.dma_start(
                        out=mu_d[r0 : r0 + p, h0 : h0 + H],
                        in_=o_t[:p, h0 : h0 + H],
                    )

    nc.compile()
    return nc, tiles


def prepare(z, size_factor, amat_w, amat_site, offsets, px_r):
    """Host-side routing + packing.  Returns (nc, in_maps, meta) where meta
    carries what unshard() needs."""
    z = np.ascontiguousarray(np.asarray(z, dtype=np.float32))
    size_factor = np.asarray(size_factor, dtype=np.float32).reshape(B)
    amat_w = np.asarray(amat_w, dtype=np.float32)
    amat_site = np.asarray(amat_site, dtype=np.float32)
    offsets = np.asarray(offsets, dtype=np.float32)
    px_r = np.asarray(px_r, dtype=np.float32)

    z0 = z[:, :L]
    ids = np.argmax(z[:, L:], axis=1)
    counts = np.bincount(ids, minlength=NE)

    # Pair experts: i-th largest with i-th smallest -> balanced core loads.
    order = np.argsort(-counts, kind="stable")
    pairs = [(int(order[i]), int(order[2 * NCORES - 1 - i])) for i in range(NCORES)]
    S0 = max(1, int(max(counts[a] for a, _ in pairs)))
    S1 = max(1, int(max(counts[b] for _, b in pairs)))
    R = S0 + S1

    # Combined per-expert weights [NE, K, O]: rows 0..63 = amat_w+amat_site[e],
    # row 64 = offsets[e] (activated by the constant-1 feature).
    w_all = np.empty((NE, K, O), dtype=np.float32)
    w_all[:, :L, :] = amat_w[None, :, :] + amat_site
    w_all[:, L, :] = offsets

    # Overflow safety: |logit| <= max_b ||[z0_b;1]|| * max_{e,o} ||W[e,:,o]||.
    # exp is finite in f32 for logits < 85; otherwise subtract the row max.
    zmax = float(np.sqrt((z0 * z0).sum(axis=1).max() + 1.0))
    wmax = float(np.sqrt((w_all * w_all).sum(axis=1).max()))
    use_max = zmax * wmax > 85.0

    key = (S0, S1, use_max, USE_F32R)
    if key not in _prog_cache:
        _prog_cache[key] = _build_program(S0, S1, use_max)
    nc, tiles = _prog_cache[key]
    T = len(tiles)
    total_cols = R + 2 * O

    in_maps = []
    core_rows = []
    pxr_in = px_r.reshape(P, O // P)
    for c in range(NCORES):
        ea, eb = pairs[c]
        rows0 = np.where(ids == ea)[0]
        rows1 = np.where(ids == eb)[0]
        core_rows.append((rows0, rows1))

        zw = np.zeros((K, total_cols), dtype=np.float32)
        zw[:L, : len(rows0)] = z0[rows0].T
        zw[L, : len(rows0)] = 1.0
        zw[:L, S0 : S0 + len(rows1)] = z0[rows1].T
        zw[L, S0 : S0 + len(rows1)] = 1.0
        zw[:, R : R + O] = w_all[ea]
        zw[:, R + O : R + 2 * O] = w_all[eb]

        sf_pad = np.ones(R, dtype=np.float32)
        sf_pad[: len(rows0)] = size_factor[rows0]
        sf_pad[S0 : S0 + len(rows1)] = size_factor[rows1]
        sfpx = np.ones((P, T + O // P), dtype=np.float32)
        for t, (r0, p, _g) in enumerate(tiles):
            sfpx[:p, t] = sf_pad[r0 : r0 + p]
        sfpx[:, T:] = pxr_in

        in_maps.append({"zw": zw, "sfpx": sfpx})

    return nc, in_maps, (core_rows, S0)


def unshard(results, meta):
    core_rows, S0 = meta
    mu = np.empty((B, O), dtype=np.float32)
    for c in range(NCORES):
        rows0, rows1 = core_rows[c]
        mu_c = results[c]["mu"]
        mu[rows0] = mu_c[: len(rows0)]
        mu[rows1] = mu_c[S0 : S0 + len(rows1)]
    theta = np.ascontiguousarray(results[0]["theta"].reshape(O))
    return mu, theta


def _ensure_axon_hooks_stub():
    """run_bass_kernel_spmd(trace=True) under axon imports antenv.axon_hooks,
    which some containers lack.  Provide a no-hook stub (only if the module is
    genuinely absent) so a BASS_TRACE=1 environment degrades to an untraced
    run instead of crashing."""
    import importlib.util
    import sys
    import types

    try:
        if importlib.util.find_spec("antenv.axon_hooks") is not None:
            return
    except (ImportError, ModuleNotFoundError):
        pass
    if "antenv.axon_hooks" in sys.modules:
        return
    m = types.ModuleType("antenv.axon_hooks")
    m.get_axon_ntff_profile_hook = lambda: None
    sys.modules["antenv.axon_hooks"] = m
    try:
        import antenv

        if not hasattr(antenv, "axon_hooks"):
            antenv.axon_hooks = m
    except ImportError:
        pass


def kernel(z, size_factor, amat_w, amat_site, offsets, px_r):
    from concourse import bass_utils

    _ensure_axon_hooks_stub()
    nc, in_maps, meta = prepare(z, size_factor, amat_w, amat_site, offsets, px_r)
    res = bass_utils.run_bass_kernel_spmd(nc, in_maps, core_ids=list(range(NCORES)))
    global last_results
    last_results = res
    return unshard(res.results, meta)
